# revision 42
# baseline (speedup 1.0000x reference)
"""BiDirectionalTriangleAttention on 8 TRN2 NeuronCores (Bass/Tile SPMD).

Sharding: I (row) axis of x1/x_pair/mask split across 8 cores (128 rows each).
Per core:
  - triangle bias tri[h, i_loc, j] = einsum(x_pair, wb) from a host-packed
    fp8-e4m3 x_pair shard in [j_half, c, i, 512] layout.  DoubleRow fp8
    matmuls carry 2 query rows per pass in the pair dim: matmul r contracts
    (c, row 2r+t) against a zero-padded lhsT whose live columns sit at
    16*h + 2r + t, so 8 accumulating matmuls fill a [128, 512] psum with
    16 rows x 8 heads at partition 16*h + i.  wb is pre-scaled x16 (fp8
    subnormal dodge); the exact 1/16 descale is folded into the score-side
    bias add / inject identity.
  - psum tiles are staged to bf16 SBUF and bounced through a packed DRAM
    scratch; per-head readback lands as tri_sb[i, h, j], where the mask bias
    16*clip(INF*(mask-1), -3600, 0) is added once (exact +0 for mask == 1).
  - the stream runs one j-half at a time so the first half's readback
    overlaps the second half's DMA; LayerNorms and all head-packed
    projections fill the PE between the halves.
  - mha_1 fully local (queries = local rows, keys = full x2n): QK via
    row-offset tile_position on head-packed q/k, tri added by DVE into the
    score psum, probs transposed by XBAR DMA for the PV matmuls.
  - mha_2 computed flash-style as a *partial* softmax over the local key
    rows (keys/values = locally updated x1u shard), emitting per-head
    unnormalized o2 partials + exp-sums (ones-augmented V) in bf16.  Host
    merges the 8 partials and applies the (tiny) gating + output projection
    + residual for x2u.
"""

import numpy as np
import ml_dtypes

import concourse.bass as bass
import concourse.bacc as bacc
import concourse.mybir as mybir
import concourse.tile as tile
from concourse.bass_utils import run_bass_kernel_spmd

F32 = mybir.dt.float32
BF16 = mybir.dt.bfloat16
FP8 = mybir.dt.float8e4
BF = ml_dtypes.bfloat16
F8 = ml_dtypes.float8_e4m3
AX = mybir.AxisListType
ALU = mybir.AluOpType
ACTF = mybir.ActivationFunctionType
DR = mybir.MatmulPerfMode.DoubleRow

B, I, J, C, H, D = 1, 1024, 1024, 128, 8, 32
HD = H * D          # 256
NCORES = 8
IS = I // NCORES    # 128 rows per core
INF = 1e9
EPS = 1e-5
ISCALE = float(1.0 / np.sqrt(np.float32(D)))
WBSC = 16.0         # host pre-scale on wb (descale via idsc inject identity)

IB = 4              # x_pair rows per psum group
IBX = 16            # x_pair rows per DMA / psum group
CP = C // 2 + 1     # 65 fp8 pair-partitions (64 data + 1 mask channel)


def _ln_tile(nc, pool, x, out_dtype, lnw_b, lnb_b, tag):
    """LayerNorm over the free (C) dim of x [P, C] -> new tile [P, C]."""
    P = x.shape[0]
    nsum = pool.tile([P, 1], F32, name=f"nsum_{tag}", tag=f"nsum_{tag}")
    nc.vector.tensor_reduce(nsum, x, axis=AX.X, op=ALU.add, negate=True)
    nc.vector.tensor_scalar_mul(nsum, nsum, 1.0 / C)          # -mu
    xc = pool.tile([P, C], F32, name=f"xc_{tag}", tag=f"xc_{tag}")
    nc.scalar.activation(xc, x, ACTF.Identity, bias=nsum, scale=1.0)  # x - mu
    sq = pool.tile([P, C], F32, name=f"sq_{tag}", tag=f"sq_{tag}")
    vs = pool.tile([P, 1], F32, name=f"vs_{tag}", tag=f"vs_{tag}")
    nc.scalar.activation(sq, xc, ACTF.Square, accum_out=vs)   # sum (x-mu)^2
    sd = pool.tile([P, 1], F32, name=f"sd_{tag}", tag=f"sd_{tag}")
    nc.scalar.activation(sd, vs, ACTF.Sqrt, bias=EPS, scale=1.0 / C)
    rstd = pool.tile([P, 1], F32, name=f"rstd_{tag}", tag=f"rstd_{tag}")
    nc.vector.reciprocal(rstd, sd)
    xn = pool.tile([P, C], F32, name=f"xn_{tag}", tag=f"xn_{tag}")
    nc.scalar.activation(xn, xc, ACTF.Copy, scale=rstd)
    nc.vector.tensor_mul(xn, xn, lnw_b)
    out = pool.tile([P, C], out_dtype, name=f"lnout_{tag}", tag=f"lnout_{tag}")
    nc.vector.tensor_add(out, xn, lnb_b)
    return out


def build_program():
    nc = bacc.Bacc("TRN2", target_bir_lowering=False, debug=False,
                   num_devices=NCORES)

    def din(name, shape, dt=F32):
        return nc.dram_tensor(name, shape, dt, kind="ExternalInput").ap()

    def dout(name, shape, dt=F32):
        return nc.dram_tensor(name, shape, dt, kind="ExternalOutput").ap()

    xpd = din("xpd", [2, C, IS, 512], FP8)  # x_pair shard, j-half major
    wbd = din("wbd", [C, 2, 8 * 128], FP8)  # 8 row-pair block-window lhsTs
    mb16 = din("mb16", [IS, J], BF16)      # 16 * clip(mask bias, -3600, 0)
    x1s = din("x1s", [IS, C])
    x2d = din("x2d", [J, C])
    # bf16 const pack: 7 x [C, HD] weights | wo1t [128, 2*C] | idbf | idsc
    wcat = din("wcat", [128, 7 * HD + 2 * C + 4 * 128], BF16)
    # f32 const pack: lnw | lnb | bg1b | id32 | bo1c
    fcat = din("fcat", [128, C + C + HD + 128 + 1])

    x1u_o = dout("x1u_o", [IS, C])
    o2p_o = dout("o2p_o", [H, D + 1, J], BF16)

    with tile.TileContext(nc) as tc:
        cst = tc.alloc_tile_pool(name="cst", bufs=1)
        sb = tc.alloc_tile_pool(name="sb", bufs=1)
        wk = tc.alloc_tile_pool(name="wk", bufs=4)
        xpp = tc.alloc_tile_pool(name="xpp", bufs=8)
        stp = tc.alloc_tile_pool(name="stp", bufs=4)
        drp = tc.alloc_tile_pool(name="drp", bufs=1, space="DRAM")
        ptri = tc.alloc_tile_pool(name="ptri", bufs=4, space="PSUM")
        ptp = tc.alloc_tile_pool(name="ptp", bufs=1, space="PSUM")
        pmm = tc.alloc_tile_pool(name="pmm", bufs=2, space="PSUM")
        pacc = tc.alloc_tile_pool(name="pacc", bufs=1, space="PSUM")

        def load(pool, ap, name, dt=None, bufs=None, eng=None):
            t = pool.tile(list(ap.shape), dt or ap.dtype, name=name, tag=name,
                          bufs=bufs)
            (eng or nc.sync).dma_start(t, ap)
            return t

        # const APs for float biases used by scalar.activation
        for cval in (0.0, EPS):
            cap = cst.tile([128, 1], F32, name=f"constap_{cval}",
                           tag=f"constap_{cval}")
            nc.vector.memset(cap, cval)
            nc.const_aps.aps[(F32, cval)] = cap

        # ---- DMA issue order on sync: wbd, smalls, consts, then the
        # x_pair stream (prefetch-deep).  Everything is packed to keep the
        # per-DMA ~0.7us issue cost off the critical path.
        c_wbd = load(cst, wbd, "c_wbd")

        NGX = IS // IBX                     # 8 tiles of 16 rows per j-half
        scr = drp.tile([2, NGX, 128, 512], BF16, name="scr", tag="scr")
        xts = {}

        def issue_xt(jh, gx):
            if gx >= NGX:
                jh, gx = jh + 1, gx - NGX
                if jh > 1:
                    return
            xt = xpp.tile([C, IBX, 512], FP8, name="xt", tag="xt")
            nc.sync.dma_start(xt, xpd[jh, :, gx * IBX:(gx + 1) * IBX, :])
            xts[(jh, gx)] = xt

        t_x1 = load(sb, x1s, "t_x1")
        t_x2 = sb.tile([128, 8, C], F32, name="t_x2", tag="t_x2")
        nc.sync.dma_start(t_x2, x2d.rearrange("(t p) c -> p t c", p=128))
        t_mb16 = load(sb, mb16, "t_mb16")
        t_wcat = load(cst, wcat, "t_wcat")
        t_fcat = load(cst, fcat, "t_fcat")
        _w = lambda k: t_wcat[:, k * HD:(k + 1) * HD]
        c_wq1t, c_wk1t, c_wv1t, c_wg1t = _w(0), _w(1), _w(2), _w(3)
        c_wq2t, c_wk2t, c_wv2t = _w(4), _w(5), _w(6)
        c_wo1t = t_wcat[:, 7 * HD:7 * HD + 2 * C].rearrange(
            "p (t c) -> p t c", t=2)
        c_idbf = t_wcat[:, 7 * HD + 2 * C:7 * HD + 2 * C + 128]
        c_idsc = t_wcat[:, 7 * HD + 2 * C + 128:7 * HD + 2 * C + 256]
        c_sel = t_wcat[:, 7 * HD + 2 * C + 256:7 * HD + 2 * C + 512]
        c_lnw = t_fcat[:, 0:C]
        c_lnb = t_fcat[:, C:2 * C]
        c_bg1b = t_fcat[:, 2 * C:2 * C + HD]
        c_id32 = t_fcat[:, 2 * C + HD:2 * C + HD + 128]
        c_bo1c = t_fcat[:, 2 * C + HD + 128:2 * C + HD + 129]

        for g in range(NGX):
            issue_xt(0, g)

        # ---- triangle bias stream: fp8 DoubleRow matmuls -> DRAM bounce ----
        # DoubleRow pair dim carries 2 query rows: matmul r contracts
        # (c, row-pair 2r+t) with a zero-padded lhsT whose live columns sit
        # at 16*h + 2r + t (head-major); 8 accumulating matmuls fill psum
        # [128, 512] with 16 rows x 8 heads at partition 16*h + i.  Streamed
        # one j-half at a time so the first half's readback + mha_1 scores
        # overlap the second half's stream.
        tri_sb = sb.tile([IS, H, J], BF16, name="tri_sb", tag="tri_sb")
        p1_all = sb.tile([IS, H, J], BF16, name="p1_all", tag="p1_all")
        l1p = sb.tile([IS, H, 2], F32, name="l1p", tag="l1p")

        def stream_half(jh):
            for gx in range(NGX):
                xt = xts.pop((jh, gx))
                ps = ptri.tile([128, 512], F32, name="ps_tri", tag="tri")
                for r in range(8):
                    nc.tensor.matmul(
                        ps, c_wbd[:, :, r * 128:(r + 1) * 128],
                        xt[:, 2 * r:2 * r + 2, :],
                        start=(r == 0), stop=False, perf_mode=DR)
                b = 32 * (gx // 2)
                bsm = slice(jh * 512, (jh + 1) * 512)
                nc.tensor.matmul(
                    ps, c_sel[b:b + 32, (gx % 2) * 128:(gx % 2) * 128 + 128],
                    t_mb16[b:b + 32, bsm],
                    start=False, stop=True, tile_position=(b, 0))
                issue_xt(jh, gx + NGX)
                stg = stp.tile([128, 512], BF16, name="stg", tag="stg")
                nc.vector.tensor_scalar_mul(stg, ps, 1.0 / WBSC)
                nc.sync.dma_start(scr[jh, gx], stg)

        def read_half(jh, eng):
            _scr_r = scr[jh].rearrange("g (h i) j -> h g i j", h=H)
            bs = slice(jh * 512, (jh + 1) * 512)
            for h in range(H):
                e = eng or (nc.sync if h % 2 == 0 else nc.scalar)
                e.dma_start(tri_sb[:, h, bs], _scr_r[h])

        stream_half(0)
        read_half(0, nc.scalar)

        # ---- LN + projections (PE work for the jh0->jh1 boundary) ----
        x1n = _ln_tile(nc, sb, t_x1, F32, c_lnw, c_lnb, "x1")
        tp = ptp.tile([128, 128], F32, name="tp_x1n", tag="tp")
        nc.tensor.transpose(tp, x1n, c_id32)
        x1nT = sb.tile([128, IS], F32, name="x1nT", tag="x1nT")
        nc.vector.tensor_copy(x1nT, tp)
        x1nTb = sb.tile([128, IS], BF16, name="x1nTb", tag="x1nTb")
        nc.scalar.copy(x1nTb, tp)

        x2nT = sb.tile([128, J], BF16, name="x2nT", tag="x2nT")
        for jt in range(8):
            x2n_jt = _ln_tile(nc, wk, t_x2[:, jt, :], BF16, c_lnw, c_lnb, "x2")
            tpb = ptp.tile([128, 128], BF16, name="tp_x2n", tag="tp")
            nc.tensor.transpose(tpb, x2n_jt, c_idbf)
            nc.vector.tensor_copy(x2nT[:, jt * 128:(jt + 1) * 128], tpb)

        # head-packed projections: partition (h % 4)*32 + d, free (h//4, seq)
        q1T = sb.tile([128, 2, IS], BF16, name="q1T", tag="q1T")
        k1T = sb.tile([128, 2, J], BF16, name="k1T", tag="k1T")
        for hf in range(2):
            cs = slice(hf * 128, (hf + 1) * 128)
            qp = pmm.tile([128, IS], F32, name="qp1", tag="mm")
            nc.tensor.matmul(qp, c_wq1t[:, cs], x1nTb, start=True, stop=True)
            nc.scalar.copy(q1T[:, hf, :], qp)
            for blk in range(2):
                kp = pmm.tile([128, 512], F32, name="kp1", tag="mm")
                nc.tensor.matmul(kp, c_wk1t[:, cs],
                                 x2nT[:, blk * 512:(blk + 1) * 512],
                                 start=True, stop=True)
                if blk == 0:
                    nc.scalar.copy(k1T[:, hf, blk * 512:(blk + 1) * 512], kp)
                else:
                    nc.vector.tensor_copy(k1T[:, hf, blk * 512:(blk + 1) * 512], kp)

        v1 = sb.tile([128, 8, HD], BF16, name="v1", tag="v1")
        for jt in range(8):
            vp = pmm.tile([128, HD], F32, name="vp1", tag="mm")
            nc.tensor.matmul(vp, x2nT[:, jt * 128:(jt + 1) * 128], c_wv1t,
                             start=True, stop=True)
            nc.vector.tensor_copy(v1[:, jt, :], vp)

        gp = pmm.tile([IS, HD], F32, name="gp1", tag="mm")
        nc.tensor.matmul(gp, x1nTb, c_wg1t, start=True, stop=True)
        g1 = sb.tile([IS, HD], F32, name="g1", tag="g1")
        nc.vector.tensor_add(g1, gp, c_bg1b)
        nc.scalar.activation(g1, g1, ACTF.Sigmoid)

        q2T = sb.tile([128, 2, J], BF16, name="q2T", tag="q2T")
        for hf in range(2):
            cs = slice(hf * 128, (hf + 1) * 128)
            for blk in range(2):
                qp2 = pmm.tile([128, 512], F32, name="qp2", tag="mm")
                nc.tensor.matmul(qp2, c_wq2t[:, cs],
                                 x2nT[:, blk * 512:(blk + 1) * 512],
                                 start=True, stop=True)
                if blk == 0:
                    nc.scalar.copy(q2T[:, hf, blk * 512:(blk + 1) * 512], qp2)
                else:
                    nc.vector.tensor_copy(
                        q2T[:, hf, blk * 512:(blk + 1) * 512], qp2)


        stream_half(1)
        read_half(1, None)

        # ---- mha_1: scores + softmax + PV, pipelined per head ----
        l1 = sb.tile([IS, H], F32, name="l1", tag="l1")
        r1 = sb.tile([IS, H], F32, name="r1", tag="r1")
        o1n = sb.tile([IS, HD], F32, name="o1n", tag="o1n")
        def mha1_scores(h, blk):
            hf, hm = h // 4, (h % 4) * 32
            bs = slice(blk * 512, (blk + 1) * 512)
            sp = ptri.tile([IS, 512], F32, name="sp1", tag="tri")
            nc.tensor.matmul(sp, q1T[hm:hm + 32, hf, :],
                             k1T[hm:hm + 32, hf, bs],
                             start=True, stop=True, tile_position=(hm, 0))
            nc.vector.tensor_add(sp, sp, tri_sb[:, h, bs])
            nc.scalar.activation(p1_all[:, h, bs], sp, ACTF.Exp,
                                 accum_out=l1p[:, h, blk:blk + 1])

        for h in range(H):
            mha1_scores(h, 0)
        for h in range(H):
            mha1_scores(h, 1)
            nc.vector.tensor_reduce(l1[:, h:h + 1], l1p[:, h, :],
                                    axis=AX.X, op=ALU.add)
            nc.vector.reciprocal(r1[:, h:h + 1], l1[:, h:h + 1])
            p1T = wk.tile([128, 8, IS], BF16, name="p1T", tag="p1T")
            nc.sync.dma_start_transpose(p1T[:, 0:4, :], p1_all[:, h, 0:512])
            nc.scalar.dma_start_transpose(p1T[:, 4:8, :], p1_all[:, h, 512:1024])
            op = pacc.tile([IS, D], F32, name="op1", tag="acc")
            for jt in range(8):
                nc.tensor.matmul(op, p1T[:, jt, :], v1[:, jt, h * D:(h + 1) * D],
                                 start=(jt == 0), stop=(jt == 7))
            nc.scalar.activation(o1n[:, h * D:(h + 1) * D], op, ACTF.Copy,
                                 scale=r1[:, h:h + 1])

        og = sb.tile([IS, HD], BF16, name="og", tag="og")
        nc.vector.tensor_mul(og, o1n, g1)
        ogT = sb.tile([128, 2, IS], BF16, name="ogT", tag="ogT")
        for t in range(2):
            tp2 = ptp.tile([128, 128], BF16, name="tp_og", tag="tp")
            nc.tensor.transpose(tp2, og[:, t * 128:(t + 1) * 128], c_idbf)
            nc.vector.tensor_copy(ogT[:, t, :], tp2)

        xop = pacc.tile([C, IS], F32, name="xop", tag="acc")
        for t in range(2):
            nc.tensor.matmul(xop, c_wo1t[:, t, :], ogT[:, t, :],
                             start=(t == 0), stop=(t == 1))
        x1uT = sb.tile([C, IS], F32, name="x1uT", tag="x1uT")
        nc.scalar.activation(x1uT, xop, ACTF.Identity, bias=c_bo1c)
        nc.vector.tensor_add(x1uT, x1uT, x1nT)

        # x1u shard out (untransposed)
        tpo = ptp.tile([128, 128], F32, name="tp_x1u", tag="tp")
        nc.tensor.transpose(tpo, x1uT, c_id32)
        x1u_sb = sb.tile([IS, C], F32, name="x1u_sb", tag="x1u_sb")
        nc.vector.tensor_copy(x1u_sb, tpo)
        nc.sync.dma_start(x1u_o, x1u_sb)

        # ---- mha_2 partials over local keys ----
        x1uTb = sb.tile([C, IS], BF16, name="x1uTb", tag="x1uTb")
        nc.scalar.copy(x1uTb, x1uT)
        k2T = sb.tile([128, 2, IS], BF16, name="k2T", tag="k2T")
        for hf in range(2):
            cs = slice(hf * 128, (hf + 1) * 128)
            kp2 = pmm.tile([128, IS], F32, name="kp2", tag="mm")
            nc.tensor.matmul(kp2, c_wk2t[:, cs], x1uTb, start=True, stop=True)
            nc.scalar.copy(k2T[:, hf, :], kp2)

        v2p = pmm.tile([IS, HD], F32, name="v2p", tag="mm")
        nc.tensor.matmul(v2p, x1uTb, c_wv2t, start=True, stop=True)
        v2a = sb.tile([IS, H, D + 1], BF16, name="v2a", tag="v2a")
        nc.vector.memset(v2a, 1.0)
        for h in range(H):
            nc.vector.tensor_copy(v2a[:, h, :D], v2p[:, h * D:(h + 1) * D])

        for h in range(H):
            hf, hm = h // 4, (h % 4) * 32
            p2 = wk.tile([IS, J], BF16, name="p2", tag="p1")
            for blk in range(2):
                bs = slice(blk * 512, (blk + 1) * 512)
                sp2 = ptri.tile([IS, 512], F32, name="sp2", tag="tri")
                nc.tensor.matmul(sp2, k2T[hm:hm + 32, hf, :],
                                 q2T[hm:hm + 32, hf, bs],
                                 start=True, stop=False, tile_position=(hm, 0))
                nc.tensor.matmul(sp2, c_idbf, tri_sb[:, h, bs],
                                 start=False, stop=True)
                nc.scalar.activation(p2[:, bs], sp2, ACTF.Exp)
            o2h = wk.tile([D + 1, J], BF16, name="o2h", tag="o2h")
            for blk in range(2):
                bs = slice(blk * 512, (blk + 1) * 512)
                o2ps = pmm.tile([D + 1, 512], F32, name="o2ps", tag="mm")
                nc.tensor.matmul(o2ps, v2a[:, h, :], p2[:, bs],
                                 start=True, stop=True)
                nc.vector.tensor_copy(o2h[:, bs], o2ps)
                nc.sync.dma_start(o2p_o[h, :, bs], o2h[:, bs])

        for p in reversed((cst, sb, wk, xpp, stp, drp, ptri, ptp, pmm, pacc)):
            p.release()

    nc.compile()
    return nc


_CACHE = {}


def _get_program():
    if "nc" not in _CACHE:
        _CACHE["nc"] = build_program()
    return _CACHE["nc"]


def _np_ln(x):
    mu = x.mean(-1, keepdims=True)
    var = np.square(x - mu).mean(-1, keepdims=True)
    return (x - mu) / np.sqrt(var + EPS)


def make_in_maps(x1, x2, x_pair, mask, ln_w, ln_b, wb,
                 wq1, wk1, wv1, wg1, bg1, wo1, bo1,
                 wq2, wk2, wv2, wg2, bg2, wo2, bo2):
    f = np.float32
    wbT = np.ascontiguousarray(np.asarray(wb, f).T)        # [C, H]
    # 4 block-window lhsTs: matmul r's lhsT (cols 128r..128r+128) is live
    # only at psum partition 16*h + 2r + t (head h of row-pair member t),
    # value 16*wb[h, c]
    wbd = np.zeros((C, 2, 8 * 128), f)
    for r in range(8):
        for t in range(2):
            for h in range(H):
                wbd[:, t, 128 * r + 16 * h + 2 * r + t] = wbT[:, h] * WBSC
    wT = lambda w: np.ascontiguousarray(np.asarray(w, f).T)

    def _sel_mask(odd):
        # sel[p, m] = 1 iff (p%32)//16 == odd and m%16 == p%16: scatters mask
        # rows (16 per half-group) into all 8 head slots of the tri psum
        p = np.arange(128)[:, None]
        m = np.arange(128)[None, :]
        return (((p % 32) // 16 == odd) & (m % 16 == p % 16)).astype(f)
    # wo1t packed as [128, 2*C]: partition p, (t, c) -> wo1.T[t*128 + p, c]
    wo1p = wT(wo1).reshape(2, 128, C).transpose(1, 0, 2).reshape(128, 2 * C)
    wcat = np.concatenate([
        wT(wq1) * ISCALE, wT(wk1), wT(wv1), wT(wg1),
        wT(wq2) * ISCALE, wT(wk2), wT(wv2),
        wo1p, np.eye(128, dtype=f), np.eye(128, dtype=f) / WBSC,
        _sel_mask(0), _sel_mask(1),
    ], axis=1)
    fcat = np.concatenate([
        np.tile(np.asarray(ln_w, f), (128, 1)),
        np.tile(np.asarray(ln_b, f), (128, 1)),
        np.tile(np.asarray(bg1, f), (128, 1)),
        np.eye(128, dtype=f),
        np.asarray(bo1, f)[:, None],
    ], axis=1)
    shared = {
        "wbd": wbd.astype(F8),
        "x2d": np.ascontiguousarray(x2[0], dtype=f),
        "wcat": wcat.astype(BF),
        "fcat": np.ascontiguousarray(fcat),
    }
    in_maps = []
    x1np = np.asarray(x1, f)
    xpnp = np.asarray(x_pair, f)
    msknp = np.asarray(mask, f)
    for m in range(NCORES):
        sl = slice(m * IS, (m + 1) * IS)
        im = dict(shared)
        im["x1s"] = np.ascontiguousarray(x1np[0, sl])
        xpc = xpnp[0, sl].transpose(2, 0, 1)               # [C, IS, J]
        im["xpd"] = np.ascontiguousarray(
            xpc.reshape(C, IS, 2, 512).transpose(2, 0, 1, 3)).astype(F8)
        mb = INF * (msknp[0, sl] - 1.0)                    # [IS, J]
        im["mb16"] = (WBSC * np.clip(mb, -3600.0, 0.0)).astype(BF)
        in_maps.append(im)
    return in_maps


def combine(results, x2, wg2, bg2, wo2, bo2):
    f = np.float32
    x1u = np.concatenate([results[m]["x1u_o"] for m in range(NCORES)],
                         axis=0)[None]
    o2p = np.sum([results[m]["o2p_o"].astype(np.float64)
                  for m in range(NCORES)], axis=0)
    o2 = o2p[:, :D, :]                    # [H, D, J]
    l2 = o2p[:, D, :]                     # [H, J]
    on = (o2 / l2[:, None, :]).astype(f)
    o_fl = on.transpose(2, 0, 1).reshape(J, HD)       # [j, hd]
    x2n = _np_ln(np.asarray(x2[0], f))
    g2 = 1.0 / (1.0 + np.exp(-(x2n @ np.asarray(wg2, f).T
                               + np.asarray(bg2, f))))
    x2u = x2n + (o_fl * g2) @ np.asarray(wo2, f).T + np.asarray(bo2, f)
    return x1u.astype(f), x2u[None].astype(f)


def kernel(**inputs):
    nc = _get_program()
    in_maps = make_in_maps(**inputs)
    res = run_bass_kernel_spmd(nc, in_maps, core_ids=list(range(NCORES)))
    return combine(res.results, inputs["x2"], inputs["wg2"], inputs["bg2"],
                   inputs["wo2"], inputs["bo2"])


if __name__ == "__main__":
    import reference
    inputs = {k: np.asarray(v) for k, v in reference.setup_inputs().items()}
    e1, e2 = reference.reference(**inputs)
    a1, a2 = kernel(**inputs)
    for name, e, a in (("x1u", e1, a1), ("x2u", e2, a2)):
        e = np.asarray(e)
        err = np.abs(a - e).max() / (np.abs(e).max() + 1e-12)
        print(f"{name}: rel_err={err:.3e}")


# revision 43
# speedup vs baseline: 1.0194x; 1.0194x over previous
"""BiDirectionalTriangleAttention on 8 TRN2 NeuronCores (Bass/Tile SPMD).

Sharding: I (row) axis of x1/x_pair/mask split across 8 cores (128 rows each).
Per core:
  - triangle bias tri[h, i_loc, j] = einsum(x_pair, wb) from a host-packed
    fp8-e4m3 x_pair shard in [j_half, c, i, 512] layout.  DoubleRow fp8
    matmuls carry 2 query rows per pass in the pair dim: matmul r contracts
    (c, row 2r+t) against a zero-padded lhsT whose live columns sit at
    16*h + 2r + t, so 8 accumulating matmuls fill a [128, 512] psum with
    16 rows x 8 heads at partition 16*h + i.  wb is pre-scaled x16 (fp8
    subnormal dodge); the exact 1/16 descale is folded into the score-side
    bias add / inject identity.
  - psum tiles are staged to bf16 SBUF and bounced through a packed DRAM
    scratch; per-head readback lands as tri_sb[i, h, j], where the mask bias
    16*clip(INF*(mask-1), -3600, 0) is added once (exact +0 for mask == 1).
  - the stream runs one j-half at a time so the first half's readback
    overlaps the second half's DMA; LayerNorms and all head-packed
    projections fill the PE between the halves.
  - mha_1 fully local (queries = local rows, keys = full x2n): QK via
    row-offset tile_position on head-packed q/k, tri added by DVE into the
    score psum, probs transposed by XBAR DMA for the PV matmuls.
  - mha_2 computed flash-style as a *partial* softmax over the local key
    rows (keys/values = locally updated x1u shard), emitting per-head
    unnormalized o2 partials + exp-sums (ones-augmented V) in bf16.  Host
    merges the 8 partials and applies the (tiny) gating + output projection
    + residual for x2u.
"""

import numpy as np
import ml_dtypes

import concourse.bass as bass
import concourse.bacc as bacc
import concourse.mybir as mybir
import concourse.tile as tile
from concourse.bass_utils import run_bass_kernel_spmd

F32 = mybir.dt.float32
BF16 = mybir.dt.bfloat16
FP8 = mybir.dt.float8e4
BF = ml_dtypes.bfloat16
F8 = ml_dtypes.float8_e4m3
AX = mybir.AxisListType
ALU = mybir.AluOpType
ACTF = mybir.ActivationFunctionType
DR = mybir.MatmulPerfMode.DoubleRow

B, I, J, C, H, D = 1, 1024, 1024, 128, 8, 32
HD = H * D          # 256
NCORES = 8
IS = I // NCORES    # 128 rows per core
INF = 1e9
EPS = 1e-5
ISCALE = float(1.0 / np.sqrt(np.float32(D)))
WBSC = 16.0         # host pre-scale on wb (descale via idsc inject identity)

IB = 4              # x_pair rows per psum group
IBX = 16            # x_pair rows per DMA / psum group
CP = C // 2 + 1     # 65 fp8 pair-partitions (64 data + 1 mask channel)


def _ln_tile(nc, pool, x, out_dtype, lnw_b, lnb_b, tag):
    """LayerNorm over the free (C) dim of x [P, C] -> new tile [P, C]."""
    P = x.shape[0]
    nsum = pool.tile([P, 1], F32, name=f"nsum_{tag}", tag=f"nsum_{tag}")
    nc.vector.tensor_reduce(nsum, x, axis=AX.X, op=ALU.add, negate=True)
    nc.vector.tensor_scalar_mul(nsum, nsum, 1.0 / C)          # -mu
    xc = pool.tile([P, C], F32, name=f"xc_{tag}", tag=f"xc_{tag}")
    nc.scalar.activation(xc, x, ACTF.Identity, bias=nsum, scale=1.0)  # x - mu
    sq = pool.tile([P, C], F32, name=f"sq_{tag}", tag=f"sq_{tag}")
    vs = pool.tile([P, 1], F32, name=f"vs_{tag}", tag=f"vs_{tag}")
    nc.scalar.activation(sq, xc, ACTF.Square, accum_out=vs)   # sum (x-mu)^2
    sd = pool.tile([P, 1], F32, name=f"sd_{tag}", tag=f"sd_{tag}")
    nc.scalar.activation(sd, vs, ACTF.Sqrt, bias=EPS, scale=1.0 / C)
    rstd = pool.tile([P, 1], F32, name=f"rstd_{tag}", tag=f"rstd_{tag}")
    nc.vector.reciprocal(rstd, sd)
    xn = pool.tile([P, C], F32, name=f"xn_{tag}", tag=f"xn_{tag}")
    nc.scalar.activation(xn, xc, ACTF.Copy, scale=rstd)
    nc.vector.tensor_mul(xn, xn, lnw_b)
    out = pool.tile([P, C], out_dtype, name=f"lnout_{tag}", tag=f"lnout_{tag}")
    nc.vector.tensor_add(out, xn, lnb_b)
    return out


def build_program():
    nc = bacc.Bacc("TRN2", target_bir_lowering=False, debug=False,
                   num_devices=NCORES)

    def din(name, shape, dt=F32):
        return nc.dram_tensor(name, shape, dt, kind="ExternalInput").ap()

    def dout(name, shape, dt=F32):
        return nc.dram_tensor(name, shape, dt, kind="ExternalOutput").ap()

    xpd = din("xpd", [2, C, IS, 512], FP8)  # x_pair shard, j-half major
    wbd = din("wbd", [C, 2, 8 * 128], FP8)  # 8 row-pair block-window lhsTs
    mb16 = din("mb16", [IS, J], BF16)      # 16 * clip(mask bias, -3600, 0)
    x1s = din("x1s", [IS, C])
    x2d = din("x2d", [J, C])
    # bf16 const pack: 7 x [C, HD] weights | wo1t [128, 2*C] | idbf | idsc
    wcat = din("wcat", [128, 7 * HD + 2 * C + 4 * 128], BF16)
    # f32 const pack: lnw | lnb | bg1b | id32 | bo1c
    fcat = din("fcat", [128, C + C + HD + 128 + 1])

    x1u_o = dout("x1u_o", [IS, C])
    o2p_o = dout("o2p_o", [H, D + 1, J], BF16)

    with tile.TileContext(nc) as tc:
        cst = tc.alloc_tile_pool(name="cst", bufs=1)
        sb = tc.alloc_tile_pool(name="sb", bufs=1)
        wk = tc.alloc_tile_pool(name="wk", bufs=4)
        xpp = tc.alloc_tile_pool(name="xpp", bufs=8)
        stp = tc.alloc_tile_pool(name="stp", bufs=4)
        drp = tc.alloc_tile_pool(name="drp", bufs=1, space="DRAM")
        ptri = tc.alloc_tile_pool(name="ptri", bufs=4, space="PSUM")
        ptp = tc.alloc_tile_pool(name="ptp", bufs=1, space="PSUM")
        pmm = tc.alloc_tile_pool(name="pmm", bufs=2, space="PSUM")
        pacc = tc.alloc_tile_pool(name="pacc", bufs=1, space="PSUM")

        def load(pool, ap, name, dt=None, bufs=None, eng=None):
            t = pool.tile(list(ap.shape), dt or ap.dtype, name=name, tag=name,
                          bufs=bufs)
            (eng or nc.sync).dma_start(t, ap)
            return t

        # const APs for float biases used by scalar.activation
        for cval in (0.0, EPS):
            cap = cst.tile([128, 1], F32, name=f"constap_{cval}",
                           tag=f"constap_{cval}")
            nc.vector.memset(cap, cval)
            nc.const_aps.aps[(F32, cval)] = cap

        # ---- DMA issue order on sync: wbd, smalls, consts, then the
        # x_pair stream (prefetch-deep).  Everything is packed to keep the
        # per-DMA ~0.7us issue cost off the critical path.
        c_wbd = load(cst, wbd, "c_wbd")

        NGX = IS // IBX                     # 8 tiles of 16 rows per j-half
        scr = drp.tile([2, NGX, 128, 512], BF16, name="scr", tag="scr")
        xts = {}

        def issue_xt(jh, gx):
            if gx >= NGX:
                jh, gx = jh + 1, gx - NGX
                if jh > 1:
                    return
            xt = xpp.tile([C, IBX, 512], FP8, name="xt", tag="xt")
            nc.sync.dma_start(xt, xpd[jh, :, gx * IBX:(gx + 1) * IBX, :])
            xts[(jh, gx)] = xt

        t_x1 = load(sb, x1s, "t_x1")
        t_x2 = sb.tile([128, 8, C], F32, name="t_x2", tag="t_x2")
        nc.sync.dma_start(t_x2, x2d.rearrange("(t p) c -> p t c", p=128))
        t_mb16 = load(sb, mb16, "t_mb16")
        t_wcat = load(cst, wcat, "t_wcat")
        t_fcat = load(cst, fcat, "t_fcat")
        _w = lambda k: t_wcat[:, k * HD:(k + 1) * HD]
        c_wq1t, c_wk1t, c_wv1t, c_wg1t = _w(0), _w(1), _w(2), _w(3)
        c_wq2t, c_wk2t, c_wv2t = _w(4), _w(5), _w(6)
        c_wo1t = t_wcat[:, 7 * HD:7 * HD + 2 * C].rearrange(
            "p (t c) -> p t c", t=2)
        c_idbf = t_wcat[:, 7 * HD + 2 * C:7 * HD + 2 * C + 128]
        c_idsc = t_wcat[:, 7 * HD + 2 * C + 128:7 * HD + 2 * C + 256]
        c_sel = t_wcat[:, 7 * HD + 2 * C + 256:7 * HD + 2 * C + 512]
        c_lnw = t_fcat[:, 0:C]
        c_lnb = t_fcat[:, C:2 * C]
        c_bg1b = t_fcat[:, 2 * C:2 * C + HD]
        c_id32 = t_fcat[:, 2 * C + HD:2 * C + HD + 128]
        c_bo1c = t_fcat[:, 2 * C + HD + 128:2 * C + HD + 129]

        for g in range(NGX):
            issue_xt(0, g)

        # ---- triangle bias stream: fp8 DoubleRow matmuls -> DRAM bounce ----
        # DoubleRow pair dim carries 2 query rows: matmul r contracts
        # (c, row-pair 2r+t) with a zero-padded lhsT whose live columns sit
        # at 16*h + 2r + t (head-major); 8 accumulating matmuls fill psum
        # [128, 512] with 16 rows x 8 heads at partition 16*h + i.  Streamed
        # one j-half at a time so the first half's readback + mha_1 scores
        # overlap the second half's stream.
        tri_sb = sb.tile([IS, H, J], BF16, name="tri_sb", tag="tri_sb")
        p1_all = sb.tile([IS, H, J], BF16, name="p1_all", tag="p1_all")
        l1p = sb.tile([IS, H, 2], F32, name="l1p", tag="l1p")

        def stream_half(jh):
            for gx in range(NGX):
                xt = xts.pop((jh, gx))
                ps = ptri.tile([128, 512], F32, name="ps_tri", tag="tri")
                for r in range(8):
                    nc.tensor.matmul(
                        ps, c_wbd[:, :, r * 128:(r + 1) * 128],
                        xt[:, 2 * r:2 * r + 2, :],
                        start=(r == 0), stop=False, perf_mode=DR)
                b = 32 * (gx // 2)
                bsm = slice(jh * 512, (jh + 1) * 512)
                nc.tensor.matmul(
                    ps, c_sel[b:b + 32, (gx % 2) * 128:(gx % 2) * 128 + 128],
                    t_mb16[b:b + 32, bsm],
                    start=False, stop=True, tile_position=(b, 0))
                issue_xt(jh, gx + NGX)
                stg = stp.tile([128, 512], BF16, name="stg", tag="stg")
                nc.vector.tensor_copy(stg, ps)
                nc.sync.dma_start(scr[jh, gx], stg)

        def read_half(jh, eng):
            _scr_r = scr[jh].rearrange("g (h i) j -> h g i j", h=H)
            bs = slice(jh * 512, (jh + 1) * 512)
            for h in range(H):
                e = eng or (nc.sync if h % 2 == 0 else nc.scalar)
                e.dma_start(tri_sb[:, h, bs], _scr_r[h])

        stream_half(0)
        read_half(0, nc.scalar)

        # ---- LN + projections (PE work for the jh0->jh1 boundary) ----
        x1n = _ln_tile(nc, sb, t_x1, F32, c_lnw, c_lnb, "x1")
        tp = ptp.tile([128, 128], F32, name="tp_x1n", tag="tp")
        nc.tensor.transpose(tp, x1n, c_id32)
        x1nT = sb.tile([128, IS], F32, name="x1nT", tag="x1nT")
        nc.vector.tensor_copy(x1nT, tp)
        x1nTb = sb.tile([128, IS], BF16, name="x1nTb", tag="x1nTb")
        nc.scalar.copy(x1nTb, tp)

        x2nT = sb.tile([128, J], BF16, name="x2nT", tag="x2nT")
        for jt in range(8):
            x2n_jt = _ln_tile(nc, wk, t_x2[:, jt, :], BF16, c_lnw, c_lnb, "x2")
            tpb = ptp.tile([128, 128], BF16, name="tp_x2n", tag="tp")
            nc.tensor.transpose(tpb, x2n_jt, c_idbf)
            nc.vector.tensor_copy(x2nT[:, jt * 128:(jt + 1) * 128], tpb)

        # head-packed projections: partition (h % 4)*32 + d, free (h//4, seq)
        q1T = sb.tile([128, 2, IS], BF16, name="q1T", tag="q1T")
        k1T = sb.tile([128, 2, J], BF16, name="k1T", tag="k1T")
        for hf in range(2):
            cs = slice(hf * 128, (hf + 1) * 128)
            qp = pmm.tile([128, IS], F32, name="qp1", tag="mm")
            nc.tensor.matmul(qp, c_wq1t[:, cs], x1nTb, start=True, stop=True)
            nc.scalar.copy(q1T[:, hf, :], qp)
            for blk in range(2):
                kp = pmm.tile([128, 512], F32, name="kp1", tag="mm")
                nc.tensor.matmul(kp, c_wk1t[:, cs],
                                 x2nT[:, blk * 512:(blk + 1) * 512],
                                 start=True, stop=True)
                if blk == 0:
                    nc.scalar.copy(k1T[:, hf, blk * 512:(blk + 1) * 512], kp)
                else:
                    nc.vector.tensor_copy(k1T[:, hf, blk * 512:(blk + 1) * 512], kp)

        v1 = sb.tile([128, 8, HD], BF16, name="v1", tag="v1")
        for jt in range(8):
            vp = pmm.tile([128, HD], F32, name="vp1", tag="mm")
            nc.tensor.matmul(vp, x2nT[:, jt * 128:(jt + 1) * 128], c_wv1t,
                             start=True, stop=True)
            nc.vector.tensor_copy(v1[:, jt, :], vp)

        gp = pmm.tile([IS, HD], F32, name="gp1", tag="mm")
        nc.tensor.matmul(gp, x1nTb, c_wg1t, start=True, stop=True)
        g1 = sb.tile([IS, HD], F32, name="g1", tag="g1")
        nc.vector.tensor_add(g1, gp, c_bg1b)
        nc.scalar.activation(g1, g1, ACTF.Sigmoid)

        q2T = sb.tile([128, 2, J], BF16, name="q2T", tag="q2T")
        for hf in range(2):
            cs = slice(hf * 128, (hf + 1) * 128)
            for blk in range(2):
                qp2 = pmm.tile([128, 512], F32, name="qp2", tag="mm")
                nc.tensor.matmul(qp2, c_wq2t[:, cs],
                                 x2nT[:, blk * 512:(blk + 1) * 512],
                                 start=True, stop=True)
                if blk == 0:
                    nc.scalar.copy(q2T[:, hf, blk * 512:(blk + 1) * 512], qp2)
                else:
                    nc.vector.tensor_copy(
                        q2T[:, hf, blk * 512:(blk + 1) * 512], qp2)


        stream_half(1)
        read_half(1, None)

        # ---- mha_1: scores + softmax + PV, pipelined per head ----
        l1 = sb.tile([IS, H], F32, name="l1", tag="l1")
        r1 = sb.tile([IS, H], F32, name="r1", tag="r1")
        o1n = sb.tile([IS, HD], F32, name="o1n", tag="o1n")
        def mha1_scores(h, blk):
            hf, hm = h // 4, (h % 4) * 32
            bs = slice(blk * 512, (blk + 1) * 512)
            sp = ptri.tile([IS, 512], F32, name="sp1", tag="tri")
            nc.tensor.matmul(sp, q1T[hm:hm + 32, hf, :],
                             k1T[hm:hm + 32, hf, bs],
                             start=True, stop=True, tile_position=(hm, 0))
            nc.vector.scalar_tensor_tensor(
                sp, tri_sb[:, h, bs], 1.0 / WBSC, sp,
                op0=ALU.mult, op1=ALU.add)
            nc.scalar.activation(p1_all[:, h, bs], sp, ACTF.Exp,
                                 accum_out=l1p[:, h, blk:blk + 1])

        for h in range(H):
            mha1_scores(h, 0)
        for h in range(H):
            mha1_scores(h, 1)
            nc.vector.tensor_reduce(l1[:, h:h + 1], l1p[:, h, :],
                                    axis=AX.X, op=ALU.add)
            nc.vector.reciprocal(r1[:, h:h + 1], l1[:, h:h + 1])
            p1T = wk.tile([128, 8, IS], BF16, name="p1T", tag="p1T")
            nc.sync.dma_start_transpose(p1T[:, 0:4, :], p1_all[:, h, 0:512])
            nc.scalar.dma_start_transpose(p1T[:, 4:8, :], p1_all[:, h, 512:1024])
            op = pacc.tile([IS, D], F32, name="op1", tag="acc")
            for jt in range(8):
                nc.tensor.matmul(op, p1T[:, jt, :], v1[:, jt, h * D:(h + 1) * D],
                                 start=(jt == 0), stop=(jt == 7))
            nc.scalar.activation(o1n[:, h * D:(h + 1) * D], op, ACTF.Copy,
                                 scale=r1[:, h:h + 1])

        og = sb.tile([IS, HD], BF16, name="og", tag="og")
        nc.vector.tensor_mul(og, o1n, g1)
        ogT = sb.tile([128, 2, IS], BF16, name="ogT", tag="ogT")
        for t in range(2):
            tp2 = ptp.tile([128, 128], BF16, name="tp_og", tag="tp")
            nc.tensor.transpose(tp2, og[:, t * 128:(t + 1) * 128], c_idbf)
            nc.vector.tensor_copy(ogT[:, t, :], tp2)

        xop = pacc.tile([C, IS], F32, name="xop", tag="acc")
        for t in range(2):
            nc.tensor.matmul(xop, c_wo1t[:, t, :], ogT[:, t, :],
                             start=(t == 0), stop=(t == 1))
        x1uT = sb.tile([C, IS], F32, name="x1uT", tag="x1uT")
        nc.scalar.activation(x1uT, xop, ACTF.Identity, bias=c_bo1c)
        nc.vector.tensor_add(x1uT, x1uT, x1nT)

        # x1u shard out (untransposed)
        tpo = ptp.tile([128, 128], F32, name="tp_x1u", tag="tp")
        nc.tensor.transpose(tpo, x1uT, c_id32)
        x1u_sb = sb.tile([IS, C], F32, name="x1u_sb", tag="x1u_sb")
        nc.vector.tensor_copy(x1u_sb, tpo)
        nc.sync.dma_start(x1u_o, x1u_sb)

        # ---- mha_2 partials over local keys ----
        x1uTb = sb.tile([C, IS], BF16, name="x1uTb", tag="x1uTb")
        nc.scalar.copy(x1uTb, x1uT)
        k2T = sb.tile([128, 2, IS], BF16, name="k2T", tag="k2T")
        for hf in range(2):
            cs = slice(hf * 128, (hf + 1) * 128)
            kp2 = pmm.tile([128, IS], F32, name="kp2", tag="mm")
            nc.tensor.matmul(kp2, c_wk2t[:, cs], x1uTb, start=True, stop=True)
            nc.scalar.copy(k2T[:, hf, :], kp2)

        v2p = pmm.tile([IS, HD], F32, name="v2p", tag="mm")
        nc.tensor.matmul(v2p, x1uTb, c_wv2t, start=True, stop=True)
        v2a = sb.tile([IS, H, D + 1], BF16, name="v2a", tag="v2a")
        nc.vector.memset(v2a, 1.0)
        for h in range(H):
            nc.vector.tensor_copy(v2a[:, h, :D], v2p[:, h * D:(h + 1) * D])

        for h in range(H):
            hf, hm = h // 4, (h % 4) * 32
            p2 = wk.tile([IS, J], BF16, name="p2", tag="p1")
            for blk in range(2):
                bs = slice(blk * 512, (blk + 1) * 512)
                sp2 = ptri.tile([IS, 512], F32, name="sp2", tag="tri")
                nc.tensor.matmul(sp2, k2T[hm:hm + 32, hf, :],
                                 q2T[hm:hm + 32, hf, bs],
                                 start=True, stop=False, tile_position=(hm, 0))
                nc.tensor.matmul(sp2, c_idsc, tri_sb[:, h, bs],
                                 start=False, stop=True)
                nc.scalar.activation(p2[:, bs], sp2, ACTF.Exp)
            o2h = wk.tile([D + 1, J], BF16, name="o2h", tag="o2h")
            for blk in range(2):
                bs = slice(blk * 512, (blk + 1) * 512)
                o2ps = pmm.tile([D + 1, 512], F32, name="o2ps", tag="mm")
                nc.tensor.matmul(o2ps, v2a[:, h, :], p2[:, bs],
                                 start=True, stop=True)
                nc.vector.tensor_copy(o2h[:, bs], o2ps)
                nc.sync.dma_start(o2p_o[h, :, bs], o2h[:, bs])

        for p in reversed((cst, sb, wk, xpp, stp, drp, ptri, ptp, pmm, pacc)):
            p.release()

    nc.compile()
    return nc


_CACHE = {}


def _get_program():
    if "nc" not in _CACHE:
        _CACHE["nc"] = build_program()
    return _CACHE["nc"]


def _np_ln(x):
    mu = x.mean(-1, keepdims=True)
    var = np.square(x - mu).mean(-1, keepdims=True)
    return (x - mu) / np.sqrt(var + EPS)


def make_in_maps(x1, x2, x_pair, mask, ln_w, ln_b, wb,
                 wq1, wk1, wv1, wg1, bg1, wo1, bo1,
                 wq2, wk2, wv2, wg2, bg2, wo2, bo2):
    f = np.float32
    wbT = np.ascontiguousarray(np.asarray(wb, f).T)        # [C, H]
    # 4 block-window lhsTs: matmul r's lhsT (cols 128r..128r+128) is live
    # only at psum partition 16*h + 2r + t (head h of row-pair member t),
    # value 16*wb[h, c]
    wbd = np.zeros((C, 2, 8 * 128), f)
    for r in range(8):
        for t in range(2):
            for h in range(H):
                wbd[:, t, 128 * r + 16 * h + 2 * r + t] = wbT[:, h] * WBSC
    wT = lambda w: np.ascontiguousarray(np.asarray(w, f).T)

    def _sel_mask(odd):
        # sel[p, m] = 1 iff (p%32)//16 == odd and m%16 == p%16: scatters mask
        # rows (16 per half-group) into all 8 head slots of the tri psum
        p = np.arange(128)[:, None]
        m = np.arange(128)[None, :]
        return (((p % 32) // 16 == odd) & (m % 16 == p % 16)).astype(f)
    # wo1t packed as [128, 2*C]: partition p, (t, c) -> wo1.T[t*128 + p, c]
    wo1p = wT(wo1).reshape(2, 128, C).transpose(1, 0, 2).reshape(128, 2 * C)
    wcat = np.concatenate([
        wT(wq1) * ISCALE, wT(wk1), wT(wv1), wT(wg1),
        wT(wq2) * ISCALE, wT(wk2), wT(wv2),
        wo1p, np.eye(128, dtype=f), np.eye(128, dtype=f) / WBSC,
        _sel_mask(0), _sel_mask(1),
    ], axis=1)
    fcat = np.concatenate([
        np.tile(np.asarray(ln_w, f), (128, 1)),
        np.tile(np.asarray(ln_b, f), (128, 1)),
        np.tile(np.asarray(bg1, f), (128, 1)),
        np.eye(128, dtype=f),
        np.asarray(bo1, f)[:, None],
    ], axis=1)
    shared = {
        "wbd": wbd.astype(F8),
        "x2d": np.ascontiguousarray(x2[0], dtype=f),
        "wcat": wcat.astype(BF),
        "fcat": np.ascontiguousarray(fcat),
    }
    in_maps = []
    x1np = np.asarray(x1, f)
    xpnp = np.asarray(x_pair, f)
    msknp = np.asarray(mask, f)
    for m in range(NCORES):
        sl = slice(m * IS, (m + 1) * IS)
        im = dict(shared)
        im["x1s"] = np.ascontiguousarray(x1np[0, sl])
        xpc = xpnp[0, sl].transpose(2, 0, 1)               # [C, IS, J]
        im["xpd"] = np.ascontiguousarray(
            xpc.reshape(C, IS, 2, 512).transpose(2, 0, 1, 3)).astype(F8)
        mb = INF * (msknp[0, sl] - 1.0)                    # [IS, J]
        im["mb16"] = (WBSC * np.clip(mb, -3600.0, 0.0)).astype(BF)
        in_maps.append(im)
    return in_maps


def combine(results, x2, wg2, bg2, wo2, bo2):
    f = np.float32
    x1u = np.concatenate([results[m]["x1u_o"] for m in range(NCORES)],
                         axis=0)[None]
    o2p = np.sum([results[m]["o2p_o"].astype(np.float64)
                  for m in range(NCORES)], axis=0)
    o2 = o2p[:, :D, :]                    # [H, D, J]
    l2 = o2p[:, D, :]                     # [H, J]
    on = (o2 / l2[:, None, :]).astype(f)
    o_fl = on.transpose(2, 0, 1).reshape(J, HD)       # [j, hd]
    x2n = _np_ln(np.asarray(x2[0], f))
    g2 = 1.0 / (1.0 + np.exp(-(x2n @ np.asarray(wg2, f).T
                               + np.asarray(bg2, f))))
    x2u = x2n + (o_fl * g2) @ np.asarray(wo2, f).T + np.asarray(bo2, f)
    return x1u.astype(f), x2u[None].astype(f)


def kernel(**inputs):
    nc = _get_program()
    in_maps = make_in_maps(**inputs)
    res = run_bass_kernel_spmd(nc, in_maps, core_ids=list(range(NCORES)))
    return combine(res.results, inputs["x2"], inputs["wg2"], inputs["bg2"],
                   inputs["wo2"], inputs["bo2"])


if __name__ == "__main__":
    import reference
    inputs = {k: np.asarray(v) for k, v in reference.setup_inputs().items()}
    e1, e2 = reference.reference(**inputs)
    a1, a2 = kernel(**inputs)
    for name, e, a in (("x1u", e1, a1), ("x2u", e2, a2)):
        e = np.asarray(e)
        err = np.abs(a - e).max() / (np.abs(e).max() + 1e-12)
        print(f"{name}: rel_err={err:.3e}")


# revision 44
# speedup vs baseline: 1.0214x; 1.0019x over previous
"""BiDirectionalTriangleAttention on 8 TRN2 NeuronCores (Bass/Tile SPMD).

Sharding: I (row) axis of x1/x_pair/mask split across 8 cores (128 rows each).
Per core:
  - triangle bias tri[h, i_loc, j] = einsum(x_pair, wb) from a host-packed
    fp8-e4m3 x_pair shard in [j_half, c, i, 512] layout.  DoubleRow fp8
    matmuls carry 2 query rows per pass in the pair dim: matmul r contracts
    (c, row 2r+t) against a zero-padded lhsT whose live columns sit at
    16*h + 2r + t, so 8 accumulating matmuls fill a [128, 512] psum with
    16 rows x 8 heads at partition 16*h + i.  wb is pre-scaled x16 (fp8
    subnormal dodge); the exact 1/16 descale is folded into the score-side
    bias add / inject identity.
  - psum tiles are staged to bf16 SBUF and bounced through a packed DRAM
    scratch; per-head readback lands as tri_sb[i, h, j], where the mask bias
    16*clip(INF*(mask-1), -3600, 0) is added once (exact +0 for mask == 1).
  - the stream runs one j-half at a time so the first half's readback
    overlaps the second half's DMA; LayerNorms and all head-packed
    projections fill the PE between the halves.
  - mha_1 fully local (queries = local rows, keys = full x2n): QK via
    row-offset tile_position on head-packed q/k, tri added by DVE into the
    score psum, probs transposed by XBAR DMA for the PV matmuls.
  - mha_2 computed flash-style as a *partial* softmax over the local key
    rows (keys/values = locally updated x1u shard), emitting per-head
    unnormalized o2 partials + exp-sums (ones-augmented V) in bf16.  Host
    merges the 8 partials and applies the (tiny) gating + output projection
    + residual for x2u.
"""

import numpy as np
import ml_dtypes

import concourse.bass as bass
import concourse.bacc as bacc
import concourse.mybir as mybir
import concourse.tile as tile
from concourse.bass_utils import run_bass_kernel_spmd

F32 = mybir.dt.float32
BF16 = mybir.dt.bfloat16
FP8 = mybir.dt.float8e4
BF = ml_dtypes.bfloat16
F8 = ml_dtypes.float8_e4m3
AX = mybir.AxisListType
ALU = mybir.AluOpType
ACTF = mybir.ActivationFunctionType
DR = mybir.MatmulPerfMode.DoubleRow

B, I, J, C, H, D = 1, 1024, 1024, 128, 8, 32
HD = H * D          # 256
NCORES = 8
IS = I // NCORES    # 128 rows per core
INF = 1e9
EPS = 1e-5
ISCALE = float(1.0 / np.sqrt(np.float32(D)))
WBSC = 16.0         # host pre-scale on wb (descale via idsc inject identity)

IB = 4              # x_pair rows per psum group
IBX = 16            # x_pair rows per DMA / psum group
CP = C // 2 + 1     # 65 fp8 pair-partitions (64 data + 1 mask channel)


def _ln_tile(nc, pool, x, out_dtype, lnw_b, lnb_b, tag):
    """LayerNorm over the free (C) dim of x [P, C] -> new tile [P, C]."""
    P = x.shape[0]
    nsum = pool.tile([P, 1], F32, name=f"nsum_{tag}", tag=f"nsum_{tag}")
    nc.vector.tensor_reduce(nsum, x, axis=AX.X, op=ALU.add, negate=True)
    nc.vector.tensor_scalar_mul(nsum, nsum, 1.0 / C)          # -mu
    xc = pool.tile([P, C], F32, name=f"xc_{tag}", tag=f"xc_{tag}")
    nc.scalar.activation(xc, x, ACTF.Identity, bias=nsum, scale=1.0)  # x - mu
    sq = pool.tile([P, C], F32, name=f"sq_{tag}", tag=f"sq_{tag}")
    vs = pool.tile([P, 1], F32, name=f"vs_{tag}", tag=f"vs_{tag}")
    nc.scalar.activation(sq, xc, ACTF.Square, accum_out=vs)   # sum (x-mu)^2
    sd = pool.tile([P, 1], F32, name=f"sd_{tag}", tag=f"sd_{tag}")
    nc.scalar.activation(sd, vs, ACTF.Sqrt, bias=EPS, scale=1.0 / C)
    rstd = pool.tile([P, 1], F32, name=f"rstd_{tag}", tag=f"rstd_{tag}")
    nc.vector.reciprocal(rstd, sd)
    xn = pool.tile([P, C], F32, name=f"xn_{tag}", tag=f"xn_{tag}")
    nc.scalar.activation(xn, xc, ACTF.Copy, scale=rstd)
    nc.vector.tensor_mul(xn, xn, lnw_b)
    out = pool.tile([P, C], out_dtype, name=f"lnout_{tag}", tag=f"lnout_{tag}")
    nc.vector.tensor_add(out, xn, lnb_b)
    return out


def build_program():
    nc = bacc.Bacc("TRN2", target_bir_lowering=False, debug=False,
                   num_devices=NCORES)

    def din(name, shape, dt=F32):
        return nc.dram_tensor(name, shape, dt, kind="ExternalInput").ap()

    def dout(name, shape, dt=F32):
        return nc.dram_tensor(name, shape, dt, kind="ExternalOutput").ap()

    xpd = din("xpd", [2, C, IS, 512], FP8)  # x_pair shard, j-half major
    wbd = din("wbd", [C, 2, 8 * 128], FP8)  # 8 row-pair block-window lhsTs
    mb16 = din("mb16", [IS, J], BF16)      # 16 * clip(mask bias, -3600, 0)
    x1s = din("x1s", [IS, C])
    x2d = din("x2d", [J, C])
    # bf16 const pack: 7 x [C, HD] weights | wo1t [128, 2*C] | idbf | idsc
    wcat = din("wcat", [128, 7 * HD + 2 * C + 4 * 128], BF16)
    # f32 const pack: lnw | lnb | bg1b | id32 | bo1c
    fcat = din("fcat", [128, C + C + HD + 128 + 1])

    x1u_o = dout("x1u_o", [IS, C])
    o2p_o = dout("o2p_o", [H, D + 1, J], BF16)

    with tile.TileContext(nc) as tc:
        cst = tc.alloc_tile_pool(name="cst", bufs=1)
        sb = tc.alloc_tile_pool(name="sb", bufs=1)
        wk = tc.alloc_tile_pool(name="wk", bufs=4)
        xpp = tc.alloc_tile_pool(name="xpp", bufs=8)
        stp = tc.alloc_tile_pool(name="stp", bufs=4)
        drp = tc.alloc_tile_pool(name="drp", bufs=1, space="DRAM")
        ptri = tc.alloc_tile_pool(name="ptri", bufs=4, space="PSUM")
        ptp = tc.alloc_tile_pool(name="ptp", bufs=1, space="PSUM")
        pmm = tc.alloc_tile_pool(name="pmm", bufs=2, space="PSUM")
        pacc = tc.alloc_tile_pool(name="pacc", bufs=1, space="PSUM")

        def load(pool, ap, name, dt=None, bufs=None, eng=None):
            t = pool.tile(list(ap.shape), dt or ap.dtype, name=name, tag=name,
                          bufs=bufs)
            (eng or nc.sync).dma_start(t, ap)
            return t

        # const APs for float biases used by scalar.activation
        for cval in (0.0, EPS):
            cap = cst.tile([128, 1], F32, name=f"constap_{cval}",
                           tag=f"constap_{cval}")
            nc.vector.memset(cap, cval)
            nc.const_aps.aps[(F32, cval)] = cap

        # ---- DMA issue order on sync: wbd, smalls, consts, then the
        # x_pair stream (prefetch-deep).  Everything is packed to keep the
        # per-DMA ~0.7us issue cost off the critical path.
        c_wbd = load(cst, wbd, "c_wbd")

        NGX = IS // IBX                     # 8 tiles of 16 rows per j-half
        scr = drp.tile([2, NGX, 128, 512], BF16, name="scr", tag="scr")
        xts = {}

        def issue_xt(jh, gx):
            if gx >= NGX:
                jh, gx = jh + 1, gx - NGX
                if jh > 1:
                    return
            xt = xpp.tile([C, IBX, 512], FP8, name="xt", tag="xt")
            nc.sync.dma_start(xt, xpd[jh, :, gx * IBX:(gx + 1) * IBX, :])
            xts[(jh, gx)] = xt

        t_x1 = load(sb, x1s, "t_x1")
        t_x2 = sb.tile([128, 8, C], F32, name="t_x2", tag="t_x2")
        nc.sync.dma_start(t_x2, x2d.rearrange("(t p) c -> p t c", p=128))
        t_mb16 = load(sb, mb16, "t_mb16")
        t_wcat = load(cst, wcat, "t_wcat")
        t_fcat = load(cst, fcat, "t_fcat")
        _w = lambda k: t_wcat[:, k * HD:(k + 1) * HD]
        c_wq1t, c_wk1t, c_wv1t, c_wg1t = _w(0), _w(1), _w(2), _w(3)
        c_wq2t, c_wk2t, c_wv2t = _w(4), _w(5), _w(6)
        c_wo1t = t_wcat[:, 7 * HD:7 * HD + 2 * C].rearrange(
            "p (t c) -> p t c", t=2)
        c_idbf = t_wcat[:, 7 * HD + 2 * C:7 * HD + 2 * C + 128]
        c_idsc = t_wcat[:, 7 * HD + 2 * C + 128:7 * HD + 2 * C + 256]
        c_sel = t_wcat[:, 7 * HD + 2 * C + 256:7 * HD + 2 * C + 512]
        c_lnw = t_fcat[:, 0:C]
        c_lnb = t_fcat[:, C:2 * C]
        c_bg1b = t_fcat[:, 2 * C:2 * C + HD]
        c_id32 = t_fcat[:, 2 * C + HD:2 * C + HD + 128]
        c_bo1c = t_fcat[:, 2 * C + HD + 128:2 * C + HD + 129]

        for g in range(NGX):
            issue_xt(0, g)

        # ---- triangle bias stream: fp8 DoubleRow matmuls -> DRAM bounce ----
        # DoubleRow pair dim carries 2 query rows: matmul r contracts
        # (c, row-pair 2r+t) with a zero-padded lhsT whose live columns sit
        # at 16*h + 2r + t (head-major); 8 accumulating matmuls fill psum
        # [128, 512] with 16 rows x 8 heads at partition 16*h + i.  Streamed
        # one j-half at a time so the first half's readback + mha_1 scores
        # overlap the second half's stream.
        tri_sb = sb.tile([IS, H, J], BF16, name="tri_sb", tag="tri_sb")
        p1_all = sb.tile([IS, H, J], BF16, name="p1_all", tag="p1_all")
        l1p = sb.tile([IS, H, 2], F32, name="l1p", tag="l1p")

        def stream_half(jh):
            for gx in range(NGX):
                xt = xts.pop((jh, gx))
                ps = ptri.tile([128, 512], F32, name="ps_tri", tag="tri")
                for r in range(8):
                    nc.tensor.matmul(
                        ps, c_wbd[:, :, r * 128:(r + 1) * 128],
                        xt[:, 2 * r:2 * r + 2, :],
                        start=(r == 0), stop=False, perf_mode=DR)
                b = 32 * (gx // 2)
                bsm = slice(jh * 512, (jh + 1) * 512)
                nc.tensor.matmul(
                    ps, c_sel[b:b + 32, (gx % 2) * 128:(gx % 2) * 128 + 128],
                    t_mb16[b:b + 32, bsm],
                    start=False, stop=True, tile_position=(b, 0))
                issue_xt(jh, gx + NGX)
                stg = stp.tile([128, 512], BF16, name="stg", tag="stg")
                nc.vector.tensor_copy(stg, ps)
                nc.sync.dma_start(scr[jh, gx], stg)

        def read_half(jh, eng):
            _scr_r = scr[jh].rearrange("g (h i) j -> h g i j", h=H)
            bs = slice(jh * 512, (jh + 1) * 512)
            for h in range(H):
                e = eng or (nc.sync if h % 2 == 0 else nc.scalar)
                e.dma_start(tri_sb[:, h, bs], _scr_r[h])

        stream_half(0)
        read_half(0, nc.scalar)

        # ---- LN + projections (PE work for the jh0->jh1 boundary) ----
        x1n = _ln_tile(nc, sb, t_x1, F32, c_lnw, c_lnb, "x1")
        tp = ptp.tile([128, 128], F32, name="tp_x1n", tag="tp")
        nc.tensor.transpose(tp, x1n, c_id32)
        x1nT = sb.tile([128, IS], F32, name="x1nT", tag="x1nT")
        nc.vector.tensor_copy(x1nT, tp)
        x1nTb = sb.tile([128, IS], BF16, name="x1nTb", tag="x1nTb")
        nc.scalar.copy(x1nTb, tp)

        x2nT = sb.tile([128, J], BF16, name="x2nT", tag="x2nT")
        for jt in range(8):
            x2n_jt = _ln_tile(nc, wk, t_x2[:, jt, :], BF16, c_lnw, c_lnb, "x2")
            tpb = ptp.tile([128, 128], BF16, name="tp_x2n", tag="tp")
            nc.tensor.transpose(tpb, x2n_jt, c_idbf)
            nc.vector.tensor_copy(x2nT[:, jt * 128:(jt + 1) * 128], tpb)

        # head-packed projections: partition (h % 4)*32 + d, free (h//4, seq)
        q1T = sb.tile([128, 2, IS], BF16, name="q1T", tag="q1T")
        k1T = sb.tile([128, 2, J], BF16, name="k1T", tag="k1T")
        for hf in range(2):
            cs = slice(hf * 128, (hf + 1) * 128)
            qp = pmm.tile([128, IS], F32, name="qp1", tag="mm")
            nc.tensor.matmul(qp, c_wq1t[:, cs], x1nTb, start=True, stop=True)
            nc.scalar.copy(q1T[:, hf, :], qp)
            for blk in range(2):
                kp = pmm.tile([128, 512], F32, name="kp1", tag="mm")
                nc.tensor.matmul(kp, c_wk1t[:, cs],
                                 x2nT[:, blk * 512:(blk + 1) * 512],
                                 start=True, stop=True)
                if blk == 0:
                    nc.scalar.copy(k1T[:, hf, blk * 512:(blk + 1) * 512], kp)
                else:
                    nc.vector.tensor_copy(k1T[:, hf, blk * 512:(blk + 1) * 512], kp)

        v1 = sb.tile([128, 8, HD], BF16, name="v1", tag="v1")
        for jt in range(8):
            vp = pmm.tile([128, HD], F32, name="vp1", tag="mm")
            nc.tensor.matmul(vp, x2nT[:, jt * 128:(jt + 1) * 128], c_wv1t,
                             start=True, stop=True)
            nc.vector.tensor_copy(v1[:, jt, :], vp)

        gp = pmm.tile([IS, HD], F32, name="gp1", tag="mm")
        nc.tensor.matmul(gp, x1nTb, c_wg1t, start=True, stop=True)
        g1 = sb.tile([IS, HD], F32, name="g1", tag="g1")
        nc.vector.tensor_add(g1, gp, c_bg1b)
        nc.scalar.activation(g1, g1, ACTF.Sigmoid)

        q2T = sb.tile([128, 2, J], BF16, name="q2T", tag="q2T")
        for hf in range(2):
            cs = slice(hf * 128, (hf + 1) * 128)
            for blk in range(2):
                qp2 = pmm.tile([128, 512], F32, name="qp2", tag="mm")
                nc.tensor.matmul(qp2, c_wq2t[:, cs],
                                 x2nT[:, blk * 512:(blk + 1) * 512],
                                 start=True, stop=True)
                if blk == 0:
                    nc.scalar.copy(q2T[:, hf, blk * 512:(blk + 1) * 512], qp2)
                else:
                    nc.vector.tensor_copy(
                        q2T[:, hf, blk * 512:(blk + 1) * 512], qp2)


        stream_half(1)
        read_half(1, None)

        # ---- mha_1: scores + softmax + PV, pipelined per head ----
        l1 = sb.tile([IS, H], F32, name="l1", tag="l1")
        r1 = sb.tile([IS, H], F32, name="r1", tag="r1")
        o1n = sb.tile([IS, HD], F32, name="o1n", tag="o1n")
        p1T_all = sb.tile([128, H, 8, IS], BF16, name="p1T_all",
                          tag="p1T_all")

        def mha1_scores(h, blk):
            hf, hm = h // 4, (h % 4) * 32
            bs = slice(blk * 512, (blk + 1) * 512)
            sp = ptri.tile([IS, 512], F32, name="sp1", tag="tri")
            nc.tensor.matmul(sp, q1T[hm:hm + 32, hf, :],
                             k1T[hm:hm + 32, hf, bs],
                             start=True, stop=True, tile_position=(hm, 0))
            nc.vector.scalar_tensor_tensor(
                sp, tri_sb[:, h, bs], 1.0 / WBSC, sp,
                op0=ALU.mult, op1=ALU.add)
            nc.scalar.activation(p1_all[:, h, bs], sp, ACTF.Exp,
                                 accum_out=l1p[:, h, blk:blk + 1])
            if blk == 0:
                nc.sync.dma_start_transpose(p1T_all[:, h, 0:4, :],
                                            p1_all[:, h, 0:512])

        for h in range(H):
            mha1_scores(h, 0)
        for h in range(H):
            mha1_scores(h, 1)
            nc.vector.tensor_reduce(l1[:, h:h + 1], l1p[:, h, :],
                                    axis=AX.X, op=ALU.add)
            nc.vector.reciprocal(r1[:, h:h + 1], l1[:, h:h + 1])
            nc.scalar.dma_start_transpose(p1T_all[:, h, 4:8, :],
                                          p1_all[:, h, 512:1024])
            op = pacc.tile([IS, D], F32, name="op1", tag="acc")
            for jt in range(8):
                nc.tensor.matmul(op, p1T_all[:, h, jt, :],
                                 v1[:, jt, h * D:(h + 1) * D],
                                 start=(jt == 0), stop=(jt == 7))
            nc.scalar.activation(o1n[:, h * D:(h + 1) * D], op, ACTF.Copy,
                                 scale=r1[:, h:h + 1])

        og = sb.tile([IS, HD], BF16, name="og", tag="og")
        nc.vector.tensor_mul(og, o1n, g1)
        ogT = sb.tile([128, 2, IS], BF16, name="ogT", tag="ogT")
        for t in range(2):
            tp2 = ptp.tile([128, 128], BF16, name="tp_og", tag="tp")
            nc.tensor.transpose(tp2, og[:, t * 128:(t + 1) * 128], c_idbf)
            nc.vector.tensor_copy(ogT[:, t, :], tp2)

        xop = pacc.tile([C, IS], F32, name="xop", tag="acc")
        for t in range(2):
            nc.tensor.matmul(xop, c_wo1t[:, t, :], ogT[:, t, :],
                             start=(t == 0), stop=(t == 1))
        x1uT = sb.tile([C, IS], F32, name="x1uT", tag="x1uT")
        nc.scalar.activation(x1uT, xop, ACTF.Identity, bias=c_bo1c)
        nc.vector.tensor_add(x1uT, x1uT, x1nT)

        # x1u shard out (untransposed)
        tpo = ptp.tile([128, 128], F32, name="tp_x1u", tag="tp")
        nc.tensor.transpose(tpo, x1uT, c_id32)
        x1u_sb = sb.tile([IS, C], F32, name="x1u_sb", tag="x1u_sb")
        nc.vector.tensor_copy(x1u_sb, tpo)
        nc.sync.dma_start(x1u_o, x1u_sb)

        # ---- mha_2 partials over local keys ----
        x1uTb = sb.tile([C, IS], BF16, name="x1uTb", tag="x1uTb")
        nc.scalar.copy(x1uTb, x1uT)
        k2T = sb.tile([128, 2, IS], BF16, name="k2T", tag="k2T")
        for hf in range(2):
            cs = slice(hf * 128, (hf + 1) * 128)
            kp2 = pmm.tile([128, IS], F32, name="kp2", tag="mm")
            nc.tensor.matmul(kp2, c_wk2t[:, cs], x1uTb, start=True, stop=True)
            nc.scalar.copy(k2T[:, hf, :], kp2)

        v2p = pmm.tile([IS, HD], F32, name="v2p", tag="mm")
        nc.tensor.matmul(v2p, x1uTb, c_wv2t, start=True, stop=True)
        v2a = sb.tile([IS, H, D + 1], BF16, name="v2a", tag="v2a")
        nc.vector.memset(v2a, 1.0)
        for h in range(H):
            nc.vector.tensor_copy(v2a[:, h, :D], v2p[:, h * D:(h + 1) * D])

        for h in range(H):
            hf, hm = h // 4, (h % 4) * 32
            p2 = wk.tile([IS, J], BF16, name="p2", tag="p1")
            for blk in range(2):
                bs = slice(blk * 512, (blk + 1) * 512)
                sp2 = ptri.tile([IS, 512], F32, name="sp2", tag="tri")
                nc.tensor.matmul(sp2, k2T[hm:hm + 32, hf, :],
                                 q2T[hm:hm + 32, hf, bs],
                                 start=True, stop=False, tile_position=(hm, 0))
                nc.tensor.matmul(sp2, c_idsc, tri_sb[:, h, bs],
                                 start=False, stop=True)
                nc.scalar.activation(p2[:, bs], sp2, ACTF.Exp)
            o2h = wk.tile([D + 1, J], BF16, name="o2h", tag="o2h")
            for blk in range(2):
                bs = slice(blk * 512, (blk + 1) * 512)
                o2ps = pmm.tile([D + 1, 512], F32, name="o2ps", tag="mm")
                nc.tensor.matmul(o2ps, v2a[:, h, :], p2[:, bs],
                                 start=True, stop=True)
                nc.vector.tensor_copy(o2h[:, bs], o2ps)
            nc.sync.dma_start(o2p_o[h], o2h)

        for p in reversed((cst, sb, wk, xpp, stp, drp, ptri, ptp, pmm, pacc)):
            p.release()

    nc.compile()
    return nc


_CACHE = {}


def _get_program():
    if "nc" not in _CACHE:
        _CACHE["nc"] = build_program()
    return _CACHE["nc"]


def _np_ln(x):
    mu = x.mean(-1, keepdims=True)
    var = np.square(x - mu).mean(-1, keepdims=True)
    return (x - mu) / np.sqrt(var + EPS)


def make_in_maps(x1, x2, x_pair, mask, ln_w, ln_b, wb,
                 wq1, wk1, wv1, wg1, bg1, wo1, bo1,
                 wq2, wk2, wv2, wg2, bg2, wo2, bo2):
    f = np.float32
    wbT = np.ascontiguousarray(np.asarray(wb, f).T)        # [C, H]
    # 4 block-window lhsTs: matmul r's lhsT (cols 128r..128r+128) is live
    # only at psum partition 16*h + 2r + t (head h of row-pair member t),
    # value 16*wb[h, c]
    wbd = np.zeros((C, 2, 8 * 128), f)
    for r in range(8):
        for t in range(2):
            for h in range(H):
                wbd[:, t, 128 * r + 16 * h + 2 * r + t] = wbT[:, h] * WBSC
    wT = lambda w: np.ascontiguousarray(np.asarray(w, f).T)

    def _sel_mask(odd):
        # sel[p, m] = 1 iff (p%32)//16 == odd and m%16 == p%16: scatters mask
        # rows (16 per half-group) into all 8 head slots of the tri psum
        p = np.arange(128)[:, None]
        m = np.arange(128)[None, :]
        return (((p % 32) // 16 == odd) & (m % 16 == p % 16)).astype(f)
    # wo1t packed as [128, 2*C]: partition p, (t, c) -> wo1.T[t*128 + p, c]
    wo1p = wT(wo1).reshape(2, 128, C).transpose(1, 0, 2).reshape(128, 2 * C)
    wcat = np.concatenate([
        wT(wq1) * ISCALE, wT(wk1), wT(wv1), wT(wg1),
        wT(wq2) * ISCALE, wT(wk2), wT(wv2),
        wo1p, np.eye(128, dtype=f), np.eye(128, dtype=f) / WBSC,
        _sel_mask(0), _sel_mask(1),
    ], axis=1)
    fcat = np.concatenate([
        np.tile(np.asarray(ln_w, f), (128, 1)),
        np.tile(np.asarray(ln_b, f), (128, 1)),
        np.tile(np.asarray(bg1, f), (128, 1)),
        np.eye(128, dtype=f),
        np.asarray(bo1, f)[:, None],
    ], axis=1)
    shared = {
        "wbd": wbd.astype(F8),
        "x2d": np.ascontiguousarray(x2[0], dtype=f),
        "wcat": wcat.astype(BF),
        "fcat": np.ascontiguousarray(fcat),
    }
    in_maps = []
    x1np = np.asarray(x1, f)
    xpnp = np.asarray(x_pair, f)
    msknp = np.asarray(mask, f)
    for m in range(NCORES):
        sl = slice(m * IS, (m + 1) * IS)
        im = dict(shared)
        im["x1s"] = np.ascontiguousarray(x1np[0, sl])
        xpc = xpnp[0, sl].transpose(2, 0, 1)               # [C, IS, J]
        im["xpd"] = np.ascontiguousarray(
            xpc.reshape(C, IS, 2, 512).transpose(2, 0, 1, 3)).astype(F8)
        mb = INF * (msknp[0, sl] - 1.0)                    # [IS, J]
        im["mb16"] = (WBSC * np.clip(mb, -3600.0, 0.0)).astype(BF)
        in_maps.append(im)
    return in_maps


def combine(results, x2, wg2, bg2, wo2, bo2):
    f = np.float32
    x1u = np.concatenate([results[m]["x1u_o"] for m in range(NCORES)],
                         axis=0)[None]
    o2p = np.sum([results[m]["o2p_o"].astype(np.float64)
                  for m in range(NCORES)], axis=0)
    o2 = o2p[:, :D, :]                    # [H, D, J]
    l2 = o2p[:, D, :]                     # [H, J]
    on = (o2 / l2[:, None, :]).astype(f)
    o_fl = on.transpose(2, 0, 1).reshape(J, HD)       # [j, hd]
    x2n = _np_ln(np.asarray(x2[0], f))
    g2 = 1.0 / (1.0 + np.exp(-(x2n @ np.asarray(wg2, f).T
                               + np.asarray(bg2, f))))
    x2u = x2n + (o_fl * g2) @ np.asarray(wo2, f).T + np.asarray(bo2, f)
    return x1u.astype(f), x2u[None].astype(f)


def kernel(**inputs):
    nc = _get_program()
    in_maps = make_in_maps(**inputs)
    res = run_bass_kernel_spmd(nc, in_maps, core_ids=list(range(NCORES)))
    return combine(res.results, inputs["x2"], inputs["wg2"], inputs["bg2"],
                   inputs["wo2"], inputs["bo2"])


if __name__ == "__main__":
    import reference
    inputs = {k: np.asarray(v) for k, v in reference.setup_inputs().items()}
    e1, e2 = reference.reference(**inputs)
    a1, a2 = kernel(**inputs)
    for name, e, a in (("x1u", e1, a1), ("x2u", e2, a2)):
        e = np.asarray(e)
        err = np.abs(a - e).max() / (np.abs(e).max() + 1e-12)
        print(f"{name}: rel_err={err:.3e}")
